# revision 7
# baseline (speedup 1.0000x reference)
"""Correlation module kernel for 8 TRN2 NeuronCores.

Reference computation (per batch element n, pure data-parallel over N):
    A_n = X_n @ U_n^T / sqrt(D)          # [L, O]
    W_n = sigmoid(A_n) - 0.5             # = 0.5 * tanh(A_n / 2)
    F_n = W_n @ U_n                      # [L, D]

Shapes: x [L=512, N=64, D=512] f32, upfold [O=512, N=64, D=512] f32.
Sharding: N axis across 8 cores (8 batch elements per core), no comms.

Device kernel (per core, per n):
    MM1:  psum_AT[o, l] = sum_d uT[d, o] * xT[d, l]      (fp16 in, f32 acc)
    ACT:  w[o, l] = tanh(psum_AT * 1/(2*sqrt(D)))        (-> fp16)
    MM2:  psum_F[l, d] = sum_o w[o, l] * (0.5*u)[o, d]   (fp16 in, f32 acc)
    DVE:  f[l, d] = psum_F                               (-> fp16)
    DMA out; host upcasts to f32 and unpacks the layout.

Timeline model (from NTFF traces, all at the full 2.4GHz clock):
  ~7us   fixed framework preamble (engine boot + barriers)
  ~3us   PE pstate ramp (427ns/matmul) -- time-based, covered by dummy
         warm-up matmuls on scratch SBUF that start as soon as the DVE
         ring can memset the scratch (~6.9us)
  55.3us 256 real matmuls at 216ns, back-to-back
  ~2.5us tail: last element's casts split DVE+ACT, stores on the two
         HWDGE rings (sync + scalar)
  ~2.6us fixed framework epilogue

Layouts: xT and uT are packed in ONE dram tensor [NLOC, P, DB, 2, 512]
(partition-major, 8KB contiguous per partition per n) so a single DMA
loads both MM1 operands; the first chunk (db0, 256KB) carries
everything MM1 needs to start. un and y are partition-major too.
"""

import numpy as np

L, O, N, D = 512, 512, 64, 512
NCORES = 8
NLOC = N // NCORES  # 8 batch elements per core
P = 128  # SBUF partitions
DB = D // P  # 4 d-blocks
OB = O // P  # 4 o-blocks
LB = L // P  # 4 l-blocks
WARMUP_MMS = 10

_cache = {}


def _build_program():
    import concourse.bass as bass
    import concourse.mybir as mybir
    import concourse.tile as tile
    from concourse import bacc

    FP16 = mybir.dt.float16
    F32 = mybir.dt.float32
    Tanh = mybir.ActivationFunctionType.Tanh
    Copy = mybir.ActivationFunctionType.Copy

    nc = bacc.Bacc("TRN2", target_bir_lowering=False, debug=False)
    # packed MM1 operands: [.., db, 0, :] = xT rows, [.., db, 1, :] = uT rows
    xu_d = nc.declare_dram_parameter("xu", [NLOC, P, DB, 2, 512], FP16, isOutput=False)
    un_d = nc.declare_dram_parameter("un", [NLOC, P, OB, D], FP16, isOutput=False)
    y_d = nc.declare_dram_parameter("y", [NLOC, P, LB, D], FP16, isOutput=True)

    s2 = 1.0 / (2.0 * float(np.sqrt(D)))  # tanh half-argument scale

    with tile.TileContext(nc) as tc:
        with (
            tc.tile_pool(name="xu", bufs=NLOC) as xu_pool,
            tc.tile_pool(name="un", bufs=NLOC) as un_pool,
            tc.tile_pool(name="w", bufs=2) as w_pool,
            tc.tile_pool(name="fo", bufs=2) as f_pool,
            tc.tile_pool(name="scr", bufs=1) as scr_pool,
            tc.tile_pool(name="psa", bufs=1, space="PSUM") as psa_pool,
            tc.tile_pool(name="psf", bufs=1, space="PSUM") as psf_pool,
        ):
            # PE warm-up on scratch SBUF: memset on the DVE ring (reaches
            # the kernel body ~1us before gpsimd) so the pstate ramp
            # starts as early as possible.
            scr_t = scr_pool.tile([P, L], FP16, tag="scr")
            nc.vector.memset(scr_t[:], 0.0)
            ps_w = psa_pool.tile([P, L], F32, tag="psa0", name="ps_warm")
            for _ in range(WARMUP_MMS):
                nc.tensor.matmul(
                    ps_w[:], lhsT=scr_t[:, :P], rhs=scr_t[:], start=True, stop=True
                )

            scr2_t = scr_pool.tile([P, 1], FP16, tag="scr2")

            for n in range(NLOC):
                xu_t = xu_pool.tile([P, DB, 2, 512], FP16, tag="xu")
                un_t = un_pool.tile([P, OB, D], FP16, tag="un")
                if n == 0:
                    # per-chunk first load in exact need order on one ring:
                    # the cold DMA window is bandwidth-bound, so arrival
                    # order must match consumption order (db0's 256KB
                    # carries both operands of MM1's first 4 matmuls)
                    nc.sync.dma_start(xu_t[:, 0, :, :], xu_d[n, :, 0, :, :])
                    nc.sync.dma_start(xu_t[:, 1, :, :], xu_d[n, :, 1, :, :])
                    nc.sync.dma_start(xu_t[:, 2:4, :, :], xu_d[n, :, 2:4, :, :])
                    # pre-trigger the ACT tanh table load (1.3us)
                    nc.scalar.activation(scr2_t[:], scr_t[:, 0:1], Tanh, scale=s2)
                else:
                    nc.sync.dma_start(xu_t[:], xu_d[n, :, :, :, :])
                nc.sync.dma_start(un_t[:], un_d[n, :, :, :])

                # -- MM1: AT[o,l] += uT.T @ xT, d-major over db0/db1 for the
                # earliest start, then o-major so each o-block's tanh
                # overlaps the remaining matmuls (n0 stays d-major so each
                # o-block closes in step with its chunk's arrival) --
                ps_a = [
                    psa_pool.tile([P, L], F32, tag=f"psa{ob}", name=f"ps_a{ob}")
                    for ob in range(OB)
                ]
                mm1_order = [(db, ob) for db in range(2) for ob in range(OB)]
                if n == 0:
                    mm1_order += [(db, ob) for db in range(2, DB) for ob in range(OB)]
                else:
                    mm1_order += [(db, ob) for ob in range(OB) for db in range(2, DB)]
                for db, ob in mm1_order:
                    nc.tensor.matmul(
                        ps_a[ob][:],
                        lhsT=xu_t[:, db, 1, bass.ts(ob, P)],
                        rhs=xu_t[:, db, 0, :],
                        start=(db == 0),
                        stop=(db == DB - 1),
                    )
                # -- sigmoid-center: w = tanh(AT * s2)  (fp16) --
                w_t = w_pool.tile([P, OB, L], FP16, tag="w")
                for ob in range(OB):
                    nc.scalar.activation(w_t[:, ob, :], ps_a[ob][:], Tanh, scale=s2)

                # -- MM2: F[l,d] += w.T @ un; o-major starts with just w[0]
                # ready; last batch element closes l-blocks early instead --
                ps_f = [
                    psf_pool.tile([P, D], F32, tag=f"psf{lb}", name=f"ps_f{lb}")
                    for lb in range(LB)
                ]
                last = n == NLOC - 1
                if last:
                    mm2_order = [(ob, lb) for lb in range(LB) for ob in range(OB)]
                else:
                    mm2_order = [(ob, lb) for ob in range(OB) for lb in range(LB)]
                for ob, lb in mm2_order:
                    nc.tensor.matmul(
                        ps_f[lb][:],
                        lhsT=w_t[:, ob, bass.ts(lb, P)],
                        rhs=un_t[:, ob, :],
                        start=(ob == 0),
                        stop=(ob == OB - 1),
                    )
                # -- PSUM -> SBUF casts split across DVE and ACT; stores on
                # the gpsimd SWDGE mid-kernel; the tail instead uses the two
                # HWDGE rings (sync + scalar) whose completion is ~1.5us
                # faster than a SWDGE drain --
                f_t = f_pool.tile([P, LB, D], FP16, tag="f")
                for lb in range(LB):
                    if last and lb == LB - 1:
                        # critical tail piece: halve the cast across DVE+ACT
                        # in parallel; store each half on its own HWDGE ring
                        # (the scalar-ring store issues right after ACT's own
                        # cast with no cross-engine semaphore hop)
                        h = D // 2
                        nc.vector.tensor_copy(f_t[:, lb, 0:h], ps_f[lb][:, 0:h])
                        nc.scalar.activation(
                            f_t[:, lb, h:D], ps_f[lb][:, h:D], Copy
                        )
                        nc.sync.dma_start(y_d[n, :, lb, 0:h], f_t[:, lb, 0:h])
                        nc.scalar.dma_start(y_d[n, :, lb, h:D], f_t[:, lb, h:D])
                        continue
                    if lb % 2 == 0:
                        nc.vector.tensor_copy(f_t[:, lb, :], ps_f[lb][:])
                    else:
                        nc.scalar.activation(f_t[:, lb, :], ps_f[lb][:], Copy)
                    if last:
                        # keep the whole tail off the slow SWDGE drain:
                        # HWDGE rings only
                        eng = nc.sync if lb % 2 == 0 else nc.scalar
                        eng.dma_start(y_d[n, :, lb, :], f_t[:, lb, :])
                    else:
                        nc.gpsimd.dma_start(y_d[n, :, lb, :], f_t[:, lb, :])
    nc.compile()
    return nc


def _prepare_in_maps(x, u):
    f16 = np.float16
    in_maps = []
    for c in range(NCORES):
        ns = slice(c * NLOC, (c + 1) * NLOC)
        xs = x[:, ns, :]  # [L, NLOC, D]
        us = u[:, ns, :]  # [O, NLOC, D]
        # X^T per n: [NLOC, D, L] -> [NLOC, P, DB, L] (partition-major)
        xt = xs.transpose(1, 2, 0).reshape(NLOC, DB, P, L).transpose(0, 2, 1, 3)
        # U^T per n: [NLOC, D, O] -> [NLOC, P, DB, O]
        ut = us.transpose(1, 2, 0).reshape(NLOC, DB, P, O).transpose(0, 2, 1, 3)
        xu = np.empty((NLOC, P, DB, 2, 512), dtype=f16)
        xu[:, :, :, 0, :] = xt
        xu[:, :, :, 1, :] = ut
        # U natural per n, pre-scaled by 0.5 (sigmoid(a)-0.5 = 0.5*tanh(a/2)):
        # [NLOC, O, D] -> [NLOC, P, OB, D] (partition-major)
        un = (
            (0.5 * us.transpose(1, 0, 2))
            .reshape(NLOC, OB, P, D)
            .transpose(0, 2, 1, 3)
            .astype(f16)
        )
        in_maps.append({"xu": np.ascontiguousarray(xu), "un": np.ascontiguousarray(un)})
    return in_maps


def _run(inputs, trace=False, **spmd_kwargs):
    from concourse.bass_utils import run_bass_kernel_spmd

    x = np.asarray(inputs["x"], dtype=np.float32)
    u = np.asarray(inputs["upfold"], dtype=np.float32)
    assert x.shape == (L, N, D) and u.shape == (O, N, D)

    if "nc" not in _cache:
        _cache["nc"] = _build_program()
    nc = _cache["nc"]

    in_maps = _prepare_in_maps(x, u)
    res = run_bass_kernel_spmd(
        nc, in_maps, core_ids=list(range(NCORES)), trace=trace, **spmd_kwargs
    )
    # y per core: [NLOC, P, LB, D] with l = lb*P + p -> [L, NLOC, D]
    outs = []
    for r in res.results:
        yc = r["y"]  # [NLOC, P, LB, D]
        outs.append(yc.transpose(2, 1, 0, 3).reshape(L, NLOC, D))
    out = np.concatenate(outs, axis=1)  # [L, N, D]
    return np.ascontiguousarray(out.astype(np.float32)), res


def kernel(**inputs) -> np.ndarray:
    out, _ = _run(inputs, trace=False)
    return out
